# revision 1
# baseline (speedup 1.0000x reference)
"""Causal self-attention with RoPE on 8 Trainium2 NeuronCores.

Problem (hardcoded): x [2, 2048, 1024] f32, w_qkv [1024, 3072], w_out [1024, 1024],
16 heads x head_dim 64, RoPE base 10000, causal softmax, out = attn @ w_out.

Sharding: DP over batch (2) x TP over head-groups (4 heads/core) = 8 cores.
Each core computes QKV for its 4 heads, full causal attention, and a partial
output projection against its 256 rows of w_out. Host sums the 4 partials per
batch element.

Per-core device layout (all matmul operands fp32r = full-rate PE):
  xT   [1024, 2048]  x[b].T
  w_qk [1024, 512]   columns: [Q h0..h3 (4*64) | K h0..h3]
  w_v  [1024, 256]
  w_o  [256, 1024]   rows of w_out for this head group
  QKT  4 tiles [128, 2048] = Q^T/K^T in [channel, seq] layout (2 heads/tile)
  Vsb  16 tiles [128, 4, 65] = V[k-tile] per head + ones column (denominator)
  scores^T strips [k=128, q<=1024] in PSUM -> exp on ACT -> P^T (fp32r)
  out^T[qc] [65, 512] PSUM accumulators: rows 0..63 = head out, row 64 = denom
"""
import numpy as np

import concourse.bacc as bacc
import concourse.tile as tile
from concourse import mybir
from concourse.bass_utils import run_bass_kernel_spmd

F32 = mybir.dt.float32
F32R = mybir.dt.float32r
EXP = mybir.ActivationFunctionType.Exp

B, S, D = 2, 2048, 1024
H, HD = 16, 64
HPC = 4          # heads per core
CQK = 2 * HPC * HD   # 512 qk channels per core
CV = HPC * HD        # 256 v channels per core
NKT = S // 128       # 16 k-tiles
NSC = S // 512       # 4 seq chunks
SCALE = 1.0 / np.sqrt(HD)
ROPE_BASE = 10000.0


def _build_nc():
    nc = bacc.Bacc(None, target_bir_lowering=False, debug=False)

    xT = nc.declare_dram_parameter("xT", [D, S], F32, isOutput=False)
    w_qk = nc.declare_dram_parameter("w_qk", [D, CQK], F32, isOutput=False)
    w_v = nc.declare_dram_parameter("w_v", [D, CV], F32, isOutput=False)
    w_o = nc.declare_dram_parameter("w_o", [CV, D], F32, isOutput=False)
    cos2 = nc.declare_dram_parameter("cos2", [128, S], F32, isOutput=False)
    sin2n = nc.declare_dram_parameter("sin2n", [128, S], F32, isOutput=False)
    umask = nc.declare_dram_parameter("umask", [128, 896], F32, isOutput=False)
    ones4 = nc.declare_dram_parameter("ones4", [128, 4, 1], F32, isOutput=False)
    out = nc.declare_dram_parameter("out", [S, D], F32, isOutput=True)

    with tile.TileContext(nc) as tc:
        with (
            tc.tile_pool(name="const", bufs=1) as const,
            tc.tile_pool(name="qkt", bufs=1) as qkt_pool,
            tc.tile_pool(name="vsb", bufs=1) as vsb_pool,
            tc.tile_pool(name="rot", bufs=3) as rot_pool,
            tc.tile_pool(name="ps", bufs=1, space="PSUM") as ps,
        ):
            # ---- constants ----
            cos_sb = const.tile([128, S], F32R)
            sin_sb = const.tile([128, S], F32R)
            u_sb = const.tile([128, 896], F32R)
            nc.sync.dma_start(out=cos_sb, in_=cos2[:, :].bitcast(F32R))
            nc.sync.dma_start(out=sin_sb, in_=sin2n[:, :].bitcast(F32R))
            nc.sync.dma_start(out=u_sb, in_=umask[:, :].bitcast(F32R))
            ones_sb = const.tile([128, 4, 1], F32R)
            nc.sync.dma_start(out=ones_sb, in_=ones4[:, :, :].bitcast(F32R))
            wo_sb = [const.tile([128, D], F32R, name=f"wo{i}") for i in range(2)]
            for i in range(2):
                nc.sync.dma_start(
                    out=wo_sb[i], in_=w_o[i * 128 : (i + 1) * 128, :].bitcast(F32R)
                )

            # persistent outputs of phase 1
            QKT = [qkt_pool.tile([128, S], F32R, name=f"qkt{t}") for t in range(4)]
            Vsb = [vsb_pool.tile([128, HPC, 65], F32R, name=f"v{k}") for k in range(NKT)]

            # ---- phase 1: QKV projection ----
            with tc.tile_pool(name="p1", bufs=1) as p1:
                wqk_sb = [p1.tile([128, CQK], F32R, name=f"wqk{d}") for d in range(8)]
                wv_sb = [p1.tile([128, CV], F32R, name=f"wv{d}") for d in range(8)]
                for d in range(8):
                    nc.sync.dma_start(
                        out=wqk_sb[d],
                        in_=w_qk[d * 128 : (d + 1) * 128, :].bitcast(F32R),
                    )
                # xT tiles chunked [d-tile 128, s-chunk 512]; sc0 first so PE starts ASAP
                xt_sb = {}
                for sc in range(NSC):
                    for d in range(8):
                        t = p1.tile([128, 512], F32R, name=f"xt{sc}_{d}")
                        nc.sync.dma_start(
                            out=t,
                            in_=xT[
                                d * 128 : (d + 1) * 128, sc * 512 : (sc + 1) * 512
                            ].bitcast(F32R),
                        )
                        xt_sb[(sc, d)] = t
                    if sc == 0:
                        for d in range(8):
                            nc.sync.dma_start(
                                out=wv_sb[d],
                                in_=w_v[d * 128 : (d + 1) * 128, :].bitcast(F32R),
                            )

                def rope(t):
                    rot = rot_pool.tile([128, S], F32R, name="rope_rot")
                    for blk in range(4):
                        sp = (blk ^ 1) * 32  # 32<->0, 96<->64 swap per head
                        nc.gpsimd.dma_start(
                            out=rot[blk * 32 : blk * 32 + 32, :],
                            in_=QKT[t][sp : sp + 32, :],
                        )
                    nc.vector.tensor_mul(rot, rot, sin_sb)
                    nc.vector.tensor_mul(QKT[t], QKT[t], cos_sb)
                    nc.vector.tensor_add(QKT[t], QKT[t], rot)

                def qk_group(ct, sc):
                    qp = ps.tile([128, 1024], F32, tag="strip", bufs=2, name=f"qk{sc}_{ct}")
                    for d in range(8):
                        nc.tensor.matmul(
                            qp[:, 0:512],
                            wqk_sb[d][:, ct * 128 : (ct + 1) * 128],
                            xt_sb[(sc, d)],
                            start=(d == 0),
                            stop=(d == 7),
                        )
                    nc.vector.tensor_copy(
                        QKT[ct][:, sc * 512 : (sc + 1) * 512], qp[:, 0:512]
                    )

                def v_group(st):
                    sc, sti = st // 4, st % 4
                    vp = ps.tile([128, 4, 64], F32, tag="strip", bufs=2, name=f"vps{st}")
                    for d in range(8):
                        nc.tensor.matmul(
                            vp,
                            xt_sb[(sc, d)][:, sti * 128 : (sti + 1) * 128],
                            wv_sb[d],
                            start=(d == 0),
                            stop=(d == 7),
                        )
                    nc.vector.tensor_copy(Vsb[st][:, :, 0:64], vp)
                    nc.vector.tensor_copy(Vsb[st][:, :, 64:65], ones_sb)

                # Q/K for heads 0,1 first, each tile roped as soon as complete;
                # V interleaved so attention heads 0/1 can start while Q/K for
                # heads 2/3 (ct 1,3) still projects.
                for ct in (0, 2):
                    for sc in range(NSC):
                        qk_group(ct, sc)
                    rope(ct)
                for st in range(NKT):
                    v_group(st)
                for ct in (1, 3):
                    for sc in range(NSC):
                        qk_group(ct, sc)
                    rope(ct)

            # ---- phase 2: attention per head ----
            with (
                tc.tile_pool(name="pp", bufs=4) as pp,
                tc.tile_pool(name="attn", bufs=1) as attn_pool,
                tc.tile_pool(name="nrm", bufs=3) as nrm,
                tc.tile_pool(name="outp", bufs=4) as outp,
            ):
                attnT = [
                    [
                        attn_pool.tile([128, 512], F32R, name=f"attnT{qc}_{ct}")
                        for ct in range(2)
                    ]
                    for qc in range(NSC)
                ]
                for h in range(HPC):
                    qt = QKT[h // 2]
                    kt_t = QKT[2 + h // 2]
                    hh = h % 2
                    outT = [
                        ps.tile([128, 512], F32, tag="outT", bufs=4, name=f"outT{h}_{qc}")
                        for qc in range(NSC)
                    ]
                    for kt in range(NKT):
                        qc0 = kt // 4
                        c0 = qc0 * 512
                        width = S - c0
                        off = c0
                        first = True
                        while off < S:
                            w = min(1024, S - off)
                            sps = ps.tile(
                                [128, 1024], F32, tag="strip", bufs=2, name=f"s{h}_{kt}_{off}"
                            )
                            for j in range(w // 512):
                                nc.tensor.matmul(
                                    sps[:, j * 512 : (j + 1) * 512],
                                    kt_t[hh * 64 : hh * 64 + 64, kt * 128 : (kt + 1) * 128],
                                    qt[hh * 64 : hh * 64 + 64, off + j * 512 : off + (j + 1) * 512],
                                    start=True,
                                    stop=True,
                                )
                            p_t = pp.tile([128, 1024], F32R, name="p_t")
                            nc.scalar.activation(
                                p_t[:, 0:w], sps[:, 0:w], EXP, scale=SCALE
                            )
                            if first:
                                r = kt % 4
                                nc.vector.tensor_mul(
                                    p_t[:, 0:512],
                                    p_t[:, 0:512],
                                    u_sb[:, 384 - 128 * r : 896 - 128 * r],
                                )
                                first = False
                            for j in range(w // 512):
                                qc = (off + j * 512) // 512
                                nc.tensor.matmul(
                                    outT[qc][0:65, :],
                                    Vsb[kt][:, h, :],
                                    p_t[:, j * 512 : (j + 1) * 512],
                                    start=(kt == 0),
                                    stop=(kt == qc * 4 + 3),
                                )
                            off += w
                        # normalize finished q-chunk (kt = qc*4+3 just stopped)
                        if kt % 4 == 3:
                            qc = qc0
                            dn = nrm.tile([1, 512], F32, name="dn")
                            nc.vector.tensor_copy(dn, outT[qc][64:65, :])
                            bc = nrm.tile([64, 512], F32, name="bc")
                            nc.gpsimd.partition_broadcast(bc, dn)
                            rc = nrm.tile([64, 512], F32, name="rc")
                            nc.vector.reciprocal(rc, bc)
                            nc.vector.tensor_mul(
                                attnT[qc][h // 2][hh * 64 : hh * 64 + 64, :],
                                outT[qc][0:64, :],
                                rc,
                            )

                # ---- phase 3: output projection ----
                for st in range(16):
                    qc = st // 4
                    sl = (st % 4) * 128
                    for ec in range(2):
                        op = ps.tile(
                            [128, 1024], F32, tag="strip", bufs=2, name=f"op{st}_{ec}"
                        )
                        for ct in range(2):
                            nc.tensor.matmul(
                                op[:, 0:512],
                                attnT[qc][ct][:, sl : sl + 128],
                                wo_sb[ct][:, ec * 512 : (ec + 1) * 512],
                                start=(ct == 0),
                                stop=(ct == 1),
                            )
                        ob = outp.tile([128, 512], F32, name="ob")
                        nc.vector.tensor_copy(ob, op[:, 0:512])
                        nc.sync.dma_start(
                            out=out[st * 128 : (st + 1) * 128, ec * 512 : (ec + 1) * 512],
                            in_=ob,
                        )
    nc.compile()
    return nc


def _host_tables():
    half = HD // 2
    inv_freq = 1.0 / (ROPE_BASE ** (np.arange(0, half, dtype=np.float64) / half))
    ang = np.arange(S, dtype=np.float64)[:, None] * inv_freq[None, :]  # [S, 32]
    cosT = np.cos(ang).T.astype(np.float32)  # [32, S]
    sinT = np.sin(ang).T.astype(np.float32)
    cos64 = np.concatenate([cosT, cosT], axis=0)  # [64, S]
    sin64s = np.concatenate([-sinT, sinT], axis=0)  # sign-folded rotate_half
    cos2 = np.ascontiguousarray(np.tile(cos64, (2, 1)))  # [128, S]
    sin2n = np.ascontiguousarray(np.tile(sin64s, (2, 1)))
    kk = np.arange(128)[:, None]
    cc = np.arange(896)[None, :]
    umask = (cc >= 384 + kk).astype(np.float32)  # [128, 896]
    return cos2, sin2n, umask


_NC_CACHE = None


def kernel(x, w_qkv, w_out):
    global _NC_CACHE
    x = np.asarray(x, dtype=np.float32)
    w_qkv = np.asarray(w_qkv, dtype=np.float32)
    w_out = np.asarray(w_out, dtype=np.float32)

    cos2, sin2n, umask = _host_tables()
    wq = w_qkv[:, 0:D]
    wk = w_qkv[:, D : 2 * D]
    wv = w_qkv[:, 2 * D : 3 * D]

    in_maps = []
    for c in range(8):
        b, hg = c // 4, c % 4
        cols = slice(hg * CV, (hg + 1) * CV)
        in_maps.append(
            {
                "xT": np.ascontiguousarray(x[b].T),
                "w_qk": np.ascontiguousarray(
                    np.concatenate([wq[:, cols], wk[:, cols]], axis=1)
                ),
                "w_v": np.ascontiguousarray(wv[:, cols]),
                "w_o": np.ascontiguousarray(w_out[cols, :]),
                "cos2": cos2,
                "sin2n": sin2n,
                "umask": umask,
                "ones4": np.ones((128, 4, 1), dtype=np.float32),
            }
        )

    if _NC_CACHE is None:
        _NC_CACHE = _build_nc()
    res = run_bass_kernel_spmd(_NC_CACHE, in_maps, core_ids=list(range(8)))
    out = np.zeros((B, S, D), dtype=np.float32)
    for c in range(8):
        out[c // 4] += res.results[c]["out"]
    return out



# revision 12
# speedup vs baseline: 1.1918x; 1.1918x over previous
"""Causal self-attention with RoPE on 8 Trainium2 NeuronCores.

Problem (hardcoded): x [2, 2048, 1024] f32, w_qkv [1024, 3072], w_out [1024, 1024],
16 heads x head_dim 64, RoPE base 10000, causal softmax, out = attn @ w_out.

Sharding: DP over batch (2) x TP over head-groups (4 heads/core) = 8 cores.
Each core computes QKV for its 4 heads, full causal attention, and a partial
output projection against its 256 rows of w_out. Host sums the 4 partials per
batch element.

Design (cost-model driven, all matmuls bf16 = 1 cyc/row):
  - RoPE via algebraic identity: roped = cosq + tan * rot(cosq), where
    cos/sin are half-symmetric so rot(cos*q) = cos*rot(q).  rot() is a +-1
    permutation matmul on PE (no shuffle DMAs).  The cos-multiply doubles as
    the PSUM->SBUF eviction of the projection.
  - Scores/P in bf16; exact-width score strips starting at the k-tile
    boundary kt*128 (causal), so the mask shrinks to one [128,128] triangle
    multiply per k-tile.
  - P@V accumulates into per-qc PSUM tiles [65,512]; row 64 is the softmax
    denominator via a ones-column in V.
  - Remaining phase-1 work (QK ct1/ct3 projection+rope, V projection) is
    interleaved into head 0's k-loop; the output projection is interleaved
    into head 3's k-loop, right after each q-chunk normalizes.
"""
import numpy as np
import ml_dtypes

import concourse.bacc as bacc
import concourse.tile as tile
from concourse import mybir
from concourse.bass_utils import run_bass_kernel_spmd

F32 = mybir.dt.float32
BF16 = mybir.dt.bfloat16
EXP = mybir.ActivationFunctionType.Exp

NP_BF16 = ml_dtypes.bfloat16

B, S, D = 2, 2048, 1024
H, HD = 16, 64
HPC = 4              # heads per core
CV = HPC * HD        # 256 v channels per core
NKT = S // 128       # 16 k-tiles
NSC = S // 512       # 4 seq chunks
SCALE = 1.0 / np.sqrt(HD)
ROPE_BASE = 10000.0


def _build_nc():
    nc = bacc.Bacc(None, target_bir_lowering=False, debug=False)

    xb8 = nc.declare_dram_parameter("xb8", [128, 8, S], BF16, isOutput=False)
    wqkb = nc.declare_dram_parameter("wqkb", [128, 8, 512], BF16, isOutput=False)
    wvb = nc.declare_dram_parameter("wvb", [128, 8, 256], BF16, isOutput=False)
    wob = nc.declare_dram_parameter("wob", [128, 2, D], BF16, isOutput=False)
    cosb = nc.declare_dram_parameter("cosb", [128, S], BF16, isOutput=False)
    tanb = nc.declare_dram_parameter("tanb", [128, S], BF16, isOutput=False)
    permb = nc.declare_dram_parameter("permb", [128, 128], BF16, isOutput=False)
    utrib = nc.declare_dram_parameter("utrib", [128, 128], BF16, isOutput=False)
    out = nc.declare_dram_parameter("out", [S, D], F32, isOutput=True)

    with tile.TileContext(nc) as tc:
        with (
            tc.tile_pool(name="const", bufs=1) as const,
            tc.tile_pool(name="qkt", bufs=1) as qkt_pool,
            tc.tile_pool(name="vsb", bufs=1) as vsb_pool,
            tc.tile_pool(name="pt", bufs=4) as pt_pool,
            tc.tile_pool(name="rope", bufs=2) as rope_pool,
            tc.tile_pool(name="attn", bufs=1) as attn_pool,
            tc.tile_pool(name="nrm", bufs=3) as nrm,
            tc.tile_pool(name="outp", bufs=2) as outp,
            tc.tile_pool(name="ps", bufs=1, space="PSUM") as ps,
        ):
            # ---- constants / inputs ----
            cos_sb = const.tile([128, S], BF16, name="cos")
            tan_sb = const.tile([128, S], BF16, name="tan")
            perm_sb = const.tile([128, 128], BF16, name="perm")
            utri_sb = const.tile([128, 128], BF16, name="utri")
            wqk_sb = const.tile([128, 8, 512], BF16, name="wqk")
            wv_sb = const.tile([128, 8, 256], BF16, name="wv")
            wo_sb = const.tile([128, 2, D], BF16, name="wo")
            xp_sb = const.tile([128, 8, S], BF16, name="xp")

            # spread input DMAs across the three DMA-capable queues
            nc.sync.dma_start(out=wqk_sb, in_=wqkb[:, :, :])
            nc.scalar.dma_start(out=perm_sb, in_=permb[:, :])
            nc.scalar.dma_start(out=cos_sb, in_=cosb[:, :])
            nc.scalar.dma_start(out=tan_sb, in_=tanb[:, :])
            for g in range(4):
                eng = (nc.sync, nc.gpsimd, nc.sync, nc.gpsimd)[g]
                eng.dma_start(
                    out=xp_sb[:, 2 * g : 2 * g + 2, :], in_=xb8[:, 2 * g : 2 * g + 2, :]
                )
            nc.gpsimd.dma_start(out=wv_sb, in_=wvb[:, :, :])
            nc.scalar.dma_start(out=wo_sb, in_=wob[:, :, :])
            nc.scalar.dma_start(out=utri_sb, in_=utrib[:, :])

            # persistent phase-1 outputs
            QKT = [qkt_pool.tile([128, S], BF16, name=f"qkt{t}") for t in range(4)]
            Vsb = [vsb_pool.tile([128, HPC, 65], BF16, name=f"v{k}") for k in range(NKT)]

            # attnT[qc]: [chan 128, ct 2, q 512], written by normalize,
            # read by the output projection
            attnT = [
                attn_pool.tile([128, 2, 512], BF16, name=f"attnT{qc}")
                for qc in range(NSC)
            ]

            def qk_group(ct, sc):
                """Project q/k channel-tile ct for seq chunk sc; fold cos."""
                qp = ps.tile([128, 512], F32, tag="strip", bufs=2, name=f"qk{ct}_{sc}")
                for d in range(8):
                    nc.tensor.matmul(
                        qp,
                        wqk_sb[:, d, ct * 128 : (ct + 1) * 128],
                        xp_sb[:, d, sc * 512 : (sc + 1) * 512],
                        start=(d == 0),
                        stop=(d == 7),
                    )
                nc.vector.tensor_mul(
                    QKT[ct][:, sc * 512 : (sc + 1) * 512],
                    qp,
                    cos_sb[:, sc * 512 : (sc + 1) * 512],
                )

            def rope(ct):
                """QKT[ct] = cosq + tan*(Perm@cosq), chunked by 512."""
                tmp = rope_pool.tile([128, S], BF16, name="ropetmp")
                for sc in range(NSC):
                    sl = slice(sc * 512, (sc + 1) * 512)
                    rot = ps.tile([128, 512], F32, tag="strip", bufs=2, name=f"rot{ct}_{sc}")
                    nc.tensor.matmul(rot, perm_sb, QKT[ct][:, sl], start=True, stop=True)
                    nc.vector.tensor_mul(tmp[:, sl], rot, tan_sb[:, sl])
                for sc in range(NSC):
                    sl = slice(sc * 512, (sc + 1) * 512)
                    nc.gpsimd.tensor_add(QKT[ct][:, sl], QKT[ct][:, sl], tmp[:, sl])

            def v_group(st):
                """Project v for seq tile st (128 positions, all 4 heads)."""
                vp = ps.tile([128, 4, 64], F32, tag="strip", bufs=2, name=f"vps{st}")
                for d in range(8):
                    nc.tensor.matmul(
                        vp,
                        xp_sb[:, d, st * 128 : (st + 1) * 128],
                        wv_sb[:, d, :],
                        start=(d == 0),
                        stop=(d == 7),
                    )
                nc.vector.tensor_copy(Vsb[st][:, :, 0:64], vp)
                nc.vector.memset(Vsb[st][:, :, 64:65], 1.0)

            def attn_head(h, pre=None, post=None):
                """Full causal attention for head h. `pre`/`post` map kt -> list
                of thunks emitted before scores / after the normalize of that
                iteration (phase-1 work for later heads, output projection)."""
                qt = QKT[h // 2]
                kt_t = QKT[2 + h // 2]
                hh = h % 2
                outT = [
                    ps.tile([128, 512], F32, tag="outT", bufs=4, name=f"o{h}_{qc}")
                    for qc in range(NSC)
                ]
                for kt in range(NKT):
                    if pre is not None:
                        for thunk in pre.get(kt, ()):
                            thunk()
                    qc0, r = kt // 4, kt % 4
                    ksl = slice(kt * 128, (kt + 1) * 128)
                    # scores strips: cover q in [kt*128, 2048) using 1024-wide
                    # PSUM tiles anchored at qc0*512
                    pts = []
                    for half in range(2):
                        base = qc0 * 512 + half * 1024
                        if base >= S:
                            break
                        wid = min(1024, S - base)
                        sps = ps.tile(
                            [128, 1024], F32, tag="strip", bufs=2, name=f"s{h}_{kt}_{half}"
                        )
                        o0 = r * 128 if half == 0 else 0
                        for j in range(0, wid, 512):
                            co = max(o0, j)
                            ce = min(j + 512, wid)
                            if co >= ce:
                                continue
                            nc.tensor.matmul(
                                sps[:, co:ce],
                                kt_t[hh * 64 : hh * 64 + 64, ksl],
                                qt[hh * 64 : hh * 64 + 64, base + co : base + ce],
                                start=True,
                                stop=True,
                            )
                        p_t = pt_pool.tile([128, 1024], BF16, name="p_t")
                        nc.scalar.activation(
                            p_t[:, o0:wid], sps[:, o0:wid], EXP, scale=SCALE
                        )
                        if half == 0:
                            nc.vector.tensor_mul(
                                p_t[:, o0 : o0 + 128],
                                p_t[:, o0 : o0 + 128],
                                utri_sb,
                            )
                        pts.append((p_t, base, o0, wid))
                    # P@V into per-qc accumulators (row 64 = denominator)
                    for p_t, base, o0, wid in pts:
                        for j in range(0, wid, 512):
                            co = max(o0, j)
                            ce = min(j + 512, wid)
                            if co >= ce:
                                continue
                            qc = (base + co) // 512
                            nc.tensor.matmul(
                                outT[qc][0:65, (base + co) % 512 : (base + co) % 512 + ce - co],
                                Vsb[kt][:, h, :],
                                p_t[:, co:ce],
                                start=(kt == 0),
                                stop=(kt == qc * 4 + 3),
                            )
                    # normalize the q-chunk completed by this diagonal k-tile
                    if r == 3:
                        qc = qc0
                        rc = nrm.tile([1, 512], F32, name="rc")
                        nc.vector.reciprocal(rc, outT[qc][64:65, :])
                        bc = nrm.tile([64, 512], F32, name="bc")
                        nc.gpsimd.partition_broadcast(bc, rc)
                        nc.vector.tensor_mul(
                            attnT[qc][hh * 64 : hh * 64 + 64, h // 2, :],
                            outT[qc][0:64, :],
                            bc,
                        )
                    if post is not None:
                        for thunk in post.get(kt, ()):
                            thunk()

            def phase3(qc):
                """Output projection for q-chunk qc."""
                for sti in range(4):
                    st = qc * 4 + sti
                    sl = slice(sti * 128, (sti + 1) * 128)
                    op = ps.tile([128, 1024], F32, tag="strip", bufs=2, name=f"op{st}")
                    for ec in range(2):
                        for ct in range(2):
                            nc.tensor.matmul(
                                op[:, ec * 512 : (ec + 1) * 512],
                                attnT[qc][:, ct, sl],
                                wo_sb[:, ct, ec * 512 : (ec + 1) * 512],
                                start=(ct == 0),
                                stop=(ct == 1),
                            )
                    ob = outp.tile([128, 1024], F32, name="ob")
                    nc.vector.tensor_copy(ob, op)
                    eng = (nc.sync, nc.gpsimd)[st % 2]
                    eng.dma_start(out=out[st * 128 : (st + 1) * 128, :], in_=ob)

            # ---- schedule ----
            for ct in (0, 2):
                for sc in range(NSC):
                    qk_group(ct, sc)
                rope(ct)
            for st in range(2):
                v_group(st)

            # interleave remaining phase-1 work into head 0's kt loop
            pre0 = {}
            for kt in range(2, NKT):
                pre0[kt] = [lambda st=kt: v_group(st)]
            pre0[2].extend([lambda: qk_group(1, 0), lambda: qk_group(1, 1)])
            pre0[3].extend([lambda: qk_group(1, 2), lambda: qk_group(1, 3)])
            pre0[4].append(lambda: rope(1))
            pre0[5].extend([lambda: qk_group(3, 0), lambda: qk_group(3, 1)])
            pre0[6].extend([lambda: qk_group(3, 2), lambda: qk_group(3, 3)])
            pre0[7].append(lambda: rope(3))

            attn_head(0, pre=pre0)
            attn_head(1)
            attn_head(2)
            # head 3: emit phase3(qc) right after its normalize closes qc
            post3 = {qc * 4 + 3: [lambda qc=qc: phase3(qc)] for qc in range(NSC)}
            attn_head(3, post=post3)
    nc.compile()
    return nc


def _host_tables():
    half = HD // 2
    inv_freq = 1.0 / (ROPE_BASE ** (np.arange(0, half, dtype=np.float64) / half))
    ang = np.arange(S, dtype=np.float64)[:, None] * inv_freq[None, :]  # [S, 32]
    cosT = np.cos(ang).T  # [32, S]
    sinT = np.sin(ang).T
    cos64 = np.concatenate([cosT, cosT], axis=0)  # [64, S]
    tan64 = np.concatenate([sinT / cosT, sinT / cosT], axis=0)
    cosb = np.tile(cos64, (2, 1))
    tanb = np.tile(tan64, (2, 1))

    # rot permutation (sign-folded): rot[c] = -x[c+32], rot[c+32] = x[c]
    # per 64-channel head block; PermT[r, c] so that rot = PermT.T @ x
    permT = np.zeros((128, 128), dtype=np.float64)
    for blk in range(2):
        o = blk * 64
        for c in range(32):
            permT[o + c + 32, o + c] = -1.0
            permT[o + c, o + c + 32] = 1.0

    kk = np.arange(128)[:, None]
    qq = np.arange(128)[None, :]
    utri = (qq >= kk).astype(np.float64)
    return (
        np.ascontiguousarray(cosb.astype(NP_BF16)),
        np.ascontiguousarray(tanb.astype(NP_BF16)),
        np.ascontiguousarray(permT.astype(NP_BF16)),
        np.ascontiguousarray(utri.astype(NP_BF16)),
    )


def _dtiles(w, d_in, width):
    """[d_in, width] -> [128, d_in//128, width] bf16 d-tiled."""
    t = w.reshape(d_in // 128, 128, width).transpose(1, 0, 2)
    return np.ascontiguousarray(t.astype(NP_BF16))


_NC_CACHE = None


def kernel(x, w_qkv, w_out):
    global _NC_CACHE
    x = np.asarray(x, dtype=np.float32)
    w_qkv = np.asarray(w_qkv, dtype=np.float32)
    w_out = np.asarray(w_out, dtype=np.float32)

    cosb, tanb, permb, utrib = _host_tables()
    wq = w_qkv[:, 0:D]
    wk = w_qkv[:, D : 2 * D]
    wv = w_qkv[:, 2 * D : 3 * D]

    in_maps = []
    for c in range(8):
        b, hg = c // 4, c % 4
        cols = slice(hg * CV, (hg + 1) * CV)
        xT = np.ascontiguousarray(x[b].T)  # [1024, 2048]
        wqk = np.concatenate([wq[:, cols], wk[:, cols]], axis=1)  # [1024, 512]
        wo = w_out[cols, :]  # [256, 1024]
        in_maps.append(
            {
                "xb8": _dtiles(xT, D, S),
                "wqkb": _dtiles(wqk, D, 512),
                "wvb": _dtiles(wv[:, cols], D, 256),
                "wob": _dtiles(wo, 256, D),
                "cosb": cosb,
                "tanb": tanb,
                "permb": permb,
                "utrib": utrib,
            }
        )

    if _NC_CACHE is None:
        _NC_CACHE = _build_nc()
    res = run_bass_kernel_spmd(_NC_CACHE, in_maps, core_ids=list(range(8)))
    out = np.zeros((B, S, D), dtype=np.float32)
    for c in range(8):
        out[c // 4] += res.results[c]["out"]
    return out
